# revision 66
# baseline (speedup 1.0000x reference)
"""Trainium2 Bass kernel for the DiscMaker mkaarma/controller scan.

Math per step t (per batch element b):
    ns    = tanh(x_t @ Wx[j] + kstate @ Wh[j])          j=0..2   [B,3,S]
    enc   = tanh(x_t @ We + kstate @ Ue)                         [B,E]
    cst   = tanh([enc, err] @ Wi + cst @ Whc)                    [B,H]
    out   = cst @ Wo                                             [B,4]
    gate  = softmax(out[:, :3] @ Wd + bd) ; theta = sigmoid(out[:, 3])
    gate  = gate*theta + gate_prev*(1-theta)
    kstate= sum_j gate[:,j] * ns[:,j,:] ; pred = kstate[:,-1] ; err = pred - y_t

Device design (per core, batch shard b=32, feature-on-partition).  The scan is
latency-bound: the serial spine per step is
    gate -> gb broadcast (PE) -> G = ns*gate (DVE) -> Ue/Wi ladder (PE/ACT)
    -> head (PE) -> exp (ACT) -> softmax blend (DVE) -> gate'
so the kernel optimizes the spine:
  - kstate never materialized: carry G[s,(j,b)] = gate[j,b]*ns[s,j,b]; all
    kstate consumers contract G with ONE matmul each whose PSUM out AP repeats
    over j (stride-0) so the 3 j-slices accumulate via has_written bits.
  - err enters the controller through PE only: Wib embedded in row 0 of a
    K=128 weight contracts G (row 0 = pred feature) straight into the
    controller PSUM; -Wib*y_t is a rank-1 matmul off the critical path.
  - gate algebra: head g-region is 6 wide [l0,l1,l2,out3,0,-big] so ONE exp
    gives [e,q=e^out3,1,0]; ONE grouped DVE reduce gives [z, w=1+q]; recip2
    gives [rz, u=1-theta] EXACTLY; then the custom DVE op GATE_ANT computes
    gate' = e*rz - (e*rz - gate_prev)*u in one instruction (no th2 tanh).
  - enc via the diag trick: PencT_j = A_ns_j^T @ Ue (PE, off-spine, PSUM in
    spare b-bank cols) -> ACT-copied to SBUF; on the spine one DVE TT builds
    diag96 = [diag(gate'_j)]x3 and three K=32 matmuls contract
    enc += PencT_j^T @ diag_j, so the old gb->G->Ue@G chain is off-spine.
  - gate head folded: Wfold = [Wo[:, :3]@Wd, 1.0*Wo[:,3]].
  - software pipelining: whc/negy/input/bdext matmuls for step t+1 are emitted
    into step t's gate-phase PE idle windows; junk matmuls fill the remaining
    PE gaps so the PE p-state stays at 2.4 GHz and the SBUF-access pipeline
    stays primed (first-matmul-after-idle costs ~185ns otherwise).
  - preds come from G[0,:] which is DMA'd out once; host sums over j.
"""

import os
import sys

import numpy as np

sys.path.insert(0, "/opt/trn_rl_repo")

import concourse.bass as bass  # noqa: E402
import concourse.tile as tile  # noqa: E402
from concourse import bacc, mybir  # noqa: E402

F16 = mybir.dt.float16
F32 = mybir.dt.float32
AF = mybir.ActivationFunctionType
ALU = mybir.AluOpType

B, T, D_IN, S, E, H, NOUT = 256, 512, 64, 128, 128, 256, 4
NCORES = 8
BC = B // NCORES  # 32 batch per core


def _junk_cols(env, default):
    v = os.environ.get(env)
    if v is None or v == "":
        return default
    if v.strip() in ("none", "-"):
        return []
    return [int(c) for c in v.split(",") if c]


FUSE_J = bool(int(os.environ.get("TRN_FUSE_J", "1")))
ERR_PE = bool(int(os.environ.get("TRN_ERR_PE", "1")))
DS_PACK = bool(int(os.environ.get("TRN_DS_PACK", "1")))
ACC_Z = bool(int(os.environ.get("TRN_ACC_Z", "0")))    # z from exp accum_out
DIV_DS = bool(int(os.environ.get("TRN_DIV_DS", "0")))  # e/z via ALU divide in ds
GATE_B = bool(int(os.environ.get("TRN_GATE_B", "1")))  # fused-gate custom DVE op
DIAG_E = GATE_B and bool(int(os.environ.get("TRN_DIAG_E", "1")))  # diag-mm enc
DIAG2 = DIAG_E and bool(int(os.environ.get("TRN_DIAG2", "0")))  # fused gatediag
KRED = bool(int(os.environ.get("TRN_KRED", "0")))  # kstate = j-reduce of G


def _register_custom_ops():
    """Register the fused gate-blend DVE ops (documented dve_ops extension
    flow, done at runtime so kernel.py stays self-contained)."""
    import concourse.dve_ops as dops
    from concourse import dve_spec as dsp
    from concourse.dve_uop import DveOpSpec
    import numpy as np_

    if "GATE_ANT" in dops._SUB_OPCODE_FOR_NAME:
        from concourse.dve_ops import OPS
        by = {o.name: o for o in OPS}
        return by["GATE_ANT"], by["GATE_DIAG_ANT"]

    def mk(name, spec):
        row = dops._CUSTOM_DVE_ROW_BASE + len(dops.OPS)
        shas = {}
        for ver in ("v3", "v4"):
            tmp = DveOpSpec(name=name, opcode=row,
                            uops=dsp.lower(spec, ver=ver),
                            rd1_en=dsp._has_src1(spec))
            shas[ver] = tmp.sha(ver)
        op = dops.DveOp(name, spec, subdim=False, uops_sha=shas)
        dops.OPS.append(op)
        dops._SUB_OPCODE_FOR_NAME[name] = row
        dops.CUSTOM_DVE_SPECS[name] = spec
        return op

    S0, S1, K0, K1 = dsp.Src0, dsp.Src1, dsp.C0, dsp.C1
    # out = sm - (sm - gp)*u; sm = e*rz.  in0=e, in1=gp, s0=rz, s1=u
    # u = 1/(1+q) is in (0,1] by construction, no clamp needed
    m1 = S0 * K0
    gate_spec = dsp.Spec(
        body=m1 - (m1 - S1) * K1,
        reference=lambda in0, in1, s0, s1, imm2:
            in0 * s0 - (in0 * s0 - in1) * s1,
    )
    # masked-diagonal variant: in1 = diag(t-1) (gp*I, off-diag exactly 0),
    # mask recovered as in1 > 0; out = diag(t) = I * (sm - (sm - gp)*u)
    msk = S1 > dsp.Zero
    mm_ = m1 * msk
    gdiag_spec = dsp.Spec(
        body=mm_ - (mm_ - S1) * K1,
        reference=lambda in0, in1, s0, s1, imm2:
            (lambda q: q - (q - in1) * s1)(in0 * s0 * (in1 > 0)),
    )
    return mk("GATE_ANT", gate_spec), mk("GATE_DIAG_ANT", gdiag_spec)
JUNK1 = _junk_cols("TRN_JUNK1", [])
JUNK2 = _junk_cols("TRN_JUNK2", [128, 128])
JUNK3 = _junk_cols("TRN_JUNK3", [128, 128])
JUNK4 = _junk_cols("TRN_JUNK4", [])


def build_program(T_steps=T):
    gate_op = gdiag_op = None
    if GATE_B:
        gate_op, gdiag_op = _register_custom_ops()
    nc = bacc.Bacc(
        "TRN2", target_bir_lowering=False, debug=False, enable_asserts=False
    )
    Tn = T_steps

    def din(name, shape, dt=F16):
        return nc.dram_tensor(name, shape, dt, kind="ExternalInput").ap()

    xw = din("xw", [S, Tn * BC])           # x[b,t,d] -> [d, t*32+b], d padded->128
    negy = din("negy", [1, Tn * BC])       # -y[b,t]  -> [1, t*32+b]
    gf20 = din("gf20", [S, 32])            # 2*gate0 in rows 0:32 cols 0:3, else 0
    whcat = din("whcat", [S, 3 * S])       # Wh[j][s_in, s_out] stacked on free
    wxcat = din("wxcat", [S, 3 * S])       # Wx[j] zero-padded rows 64:128
    ue = din("ue", [S, E])
    we = din("we", [S, E])                 # We zero-padded rows 64:128
    wit0 = din("wit0", [E, S])             # Wi[0:128] cols 0:128
    wit1 = din("wit1", [E, S])             # Wi[0:128] cols 128:256
    wib = din("wib", [1, H])               # Wi[128]
    wibr0 = din("wibr0", [S, H])           # Wi[128] embedded in row 0, else 0
    whc00 = din("whc00", [S, S])           # Whc[0:128, 0:128]
    whc01 = din("whc01", [S, S])           # Whc[0:128, 128:256]
    whc10 = din("whc10", [S, S])           # Whc[128:256, 0:128]
    whc11 = din("whc11", [S, S])           # Whc[128:256, 128:256]
    GW = 6 if GATE_B else 4                # gate-head region width
    wfold = din("wfold", [S, 8])           # [F[0:128] | F[128:256]], F=[Wo@Wd_ext]
    bdext = din("bdext", [1, GW])          # [bd, 0] (+ [0, -big] for exp6)
    eye32 = din("eye32", [S, BC])          # 0.5*I_32 in rows 0:32, zeros below
    ones132 = din("ones132", [1, S])       # 1.0 in cols 0:32, 0 elsewhere
    if DIAG_E:
        eyem96 = din("eyem96", [BC, 96])   # [I32|I32|I32]
    if DIAG2:
        diag0 = din("diag0", [BC, 96])     # [diag(gate0_j)]x3

    g127o = nc.dram_tensor("g127", [1, Tn * 96], F16, kind="ExternalOutput").ap()

    with tile.TileContext(nc) as tc:
        import contextlib
        stk = contextlib.ExitStack()
        persist = stk.enter_context(tc.tile_pool(name="persist", bufs=1))

        def ptile(shape, dtype, name):
            return persist.tile(shape, dtype, name=name, tag=name)

        # ---- persistent SBUF ----
        s_xw = ptile([S, Tn * BC], F16, "s_xw")
        s_negy = ptile([1, Tn * BC], F16, "s_negy")
        s_whcat = ptile([S, 3 * S], F16, "s_whcat")
        s_wxcat = ptile([S, 3 * S], F16, "s_wxcat")
        s_ue = ptile([S, E], F16, "s_ue")
        s_we = ptile([S, E], F16, "s_we")
        s_wit = [ptile([E, S], F16, "s_wit0"), ptile([E, S], F16, "s_wit1")]
        s_wib = ptile([1, H], F16, "s_wib")
        s_wibr0 = ptile([S, H], F16, "s_wibr0")
        s_whc = [[ptile([S, S], F16, "s_whc00"), ptile([S, S], F16, "s_whc01")],
                 [ptile([S, S], F16, "s_whc10"), ptile([S, S], F16, "s_whc11")]]
        s_wfold = ptile([S, 8], F16, "s_wfold")
        s_bdext = ptile([1, GW], F16, "s_bdext")
        s_eye = ptile([S, BC], F16, "s_eye")
        s_ones132 = ptile([1, S], F16, "s_ones132")
        if DIAG_E:
            s_eyem = ptile([BC, 96], F16, "s_eyem")
            s_pencT = ptile([BC, 3 * E], F16, "s_pencT")
        if DIAG2:
            diagA = ptile([BC, 96], F16, "diagA")
            diagB = ptile([BC, 96], F16, "diagB")
            dpar = [diagA, diagB]
            s_one32 = ptile([BC, 1], F16, "s_one32")
        s_gall = ptile([S, Tn * 96], F16, "s_gall")
        s_cstA = ptile([S, 5 * BC], F16, "s_cstA")
        s_cstB = ptile([S, 5 * BC], F16, "s_cstB")
        s_cst2 = [s_cstA, s_cstB]
        gf2a = ptile([S, 32], F16, "gf2a")
        gf2b = ptile([S, 32], F16, "gf2b")
        s_z1 = ptile([S, 1], F16, "s_z1")
        chalf = ptile([BC, 6], F32, "chalf")
        hgp2a = ptile([BC, 6], F32, "hgp2a")
        hgp2b = ptile([BC, 6], F32, "hgp2b")

        # DMA order = need order: step-0's inputs first (xw slice 0, We/Wx,
        # head consts, gate0), then the rest, then the bulk xw slices. The
        # DMA stream is serial-ish, so step 0 starts ~15us earlier this way.
        NXCH = 32
        xch = Tn * BC // NXCH
        nc.sync.dma_start(out=s_xw[:, 0:xch], in_=xw[:, 0:xch])
        for dst, src in [
            (s_we, we), (s_wxcat, wxcat), (s_ones132, ones132),
            (s_bdext, bdext), (gf2a, gf20),
            (s_wit[0], wit0), (s_wit[1], wit1), (s_ue, ue),
            (s_wfold, wfold), (s_eye, eye32),
        ] + ([(s_eyem, eyem96)] if DIAG_E else []) \
          + ([(diagA, diag0)] if DIAG2 else []) + [
            (s_whcat, whcat), (s_negy, negy), (s_wib, wib), (s_wibr0, wibr0),
            (s_whc[0][0], whc00), (s_whc[0][1], whc01),
            (s_whc[1][0], whc10), (s_whc[1][1], whc11),
        ]:
            nc.sync.dma_start(out=dst[:], in_=src)
        for c in range(1, NXCH):
            nc.sync.dma_start(out=s_xw[:, c * xch:(c + 1) * xch],
                              in_=xw[:, c * xch:(c + 1) * xch])
        nc.vector.memset(gf2b[:], 0.0)
        nc.vector.memset(s_z1[:], 0.0)
        if DIAG2:
            nc.vector.memset(s_one32[:], 1.0)
            nc.vector.memset(diagB[:], 0.0)
        nc.vector.memset(s_cstA[:, 2 * BC:5 * BC], 0.0)
        nc.vector.memset(s_cstB[:, 2 * BC:5 * BC], 0.0)
        if not GATE_B:
            nc.vector.memset(chalf[:, 0:3], -0.5)
            nc.vector.memset(chalf[:, 3:6], 0.5)
            # hgp2 = [-gate | +gate] = gf2 * [-0.5|+0.5]  (gf2 carries 2*gate)
            c3 = chalf[:].rearrange("p (a b) -> p a b", a=2)
            nc.vector.tensor_tensor(
                hgp2a[:].rearrange("p (a b) -> p a b", a=2),
                gf2a[0:BC, 0:3].unsqueeze(1).broadcast_to([BC, 2, 3]),
                c3, ALU.mult)

        # ---- pools ----
        # PSUM is 8 banks of 2KB/partition; zero regions (start=True scope)
        # are bank-sized, so each bank gets exactly ONE start=True per step
        # (the first write); everything else store-on-first-touch/accumulates.
        # The tiny gate-head tile g shares pEnc's bank (cols 32:36).
        pEnc = stk.enter_context(tc.tile_pool(name="pEnc", bufs=2, space="PSUM"))
        pNs = stk.enter_context(tc.tile_pool(name="pNs", bufs=2, space="PSUM"))
        pB = stk.enter_context(tc.tile_pool(name="pB", bufs=2, space="PSUM"))
        pGB = stk.enter_context(tc.tile_pool(name="pGB", bufs=1, space="PSUM"))
        pJ = stk.enter_context(tc.tile_pool(name="pJ", bufs=1, space="PSUM"))
        wk = stk.enter_context(tc.tile_pool(name="wk", bufs=3))

        ts = bass.ts
        gf2 = [gf2a, gf2b]
        hgp2 = [hgp2a, hgp2b]
        mm = nc.tensor.matmul

        jt = pJ.tile([S, 512], F32, name="jt", tag="junk")
        jrot = [0]

        def junk(cols_list, rhs1=None):
            # dependency-gated junk: rhs1 (a [K,1] fp16 SBUF AP) delays
            # readiness until its producer finishes, so the junk fills a
            # specific pipeline gap instead of greedily running early.
            # Output regions rotate across 4 slices of the junk bank so the
            # WAW dependency (sem fires ~173ns after exec) is 4 junks back
            # and never stalls the junk stream.
            for c in cols_list:
                c = min(c, 128)
                off = (jrot[0] % 4) * 128
                jrot[0] += 1
                if rhs1 is None:
                    lhsT, rhs = s_whcat[:, 0:S], s_whcat[:, 0:c]
                else:
                    kp = rhs1.partition_size()
                    lhsT = s_whcat[0:kp, 0:S]
                    rhs = rhs1.broadcast_to([kp, c])
                mm(jt[:, off:off + c], lhsT, rhs,
                   start=True, stop=True, skip_group_check=True)

        def bj(ap):  # [S, n] -> [S, 3, n] with stride-0 j (PSUM accumulate)
            return ap.unsqueeze(1).broadcast_to([ap.shape[0], 3, ap.shape[1]])

        # ---- prologue: step-0 input matmuls ----
        xt0 = s_xw[:, 0:BC]
        eg_cur = pEnc.tile([S, 512], F32, tag="a_enc")
        enc_cur = eg_cur[:, 0:BC]
        g_cur = eg_cur[:, BC:BC + GW]
        mm(enc_cur, s_we[:], xt0, start=True, stop=True)
        mm(g_cur, s_ones132[:], s_bdext[:], start=False, stop=False)
        ns_cur = pNs.tile([S, 512], F32, name="a_ns", tag="a_ns")[:, 0:96]
        for k in range(3):
            mm(ns_cur[:, ts(k, BC)], s_wxcat[:, ts(k, S)], xt0,
               start=(k == 0), stop=(k == 2))
        b_full = pB.tile([S, 512], F32, name="b_", tag="b_")
        b_cur = b_full[:, 0:2 * BC]

        for t in range(Tn):
            first = (t == 0)
            last = (t == Tn - 1)
            gprev = None if first else s_gall[:, ts(t - 1, 96)]
            gn = gf2[(t + 1) % 2]

            # ---- [PE] G-dependent accumulations (spine head) ----
            if not first:
                if DIAG_E:
                    pass  # enc kstate-term comes from diag-mms of step t-1
                elif FUSE_J:
                    mm(bj(enc_cur), s_ue[:], gprev, start=False, stop=False)
                else:
                    for j in range(3):
                        mm(enc_cur, s_ue[:], gprev[:, ts(j, BC)],
                           start=False, stop=False)
                if ERR_PE and KRED:
                    # Wib x pred via row-0-embedded weight contracting kstate
                    for h in range(2):
                        mm(b_cur[:, ts(h, BC)], s_wibr0[:, ts(h, S)],
                           kprev[:], start=False, stop=False)
                elif ERR_PE:
                    # Wib x pred: row-0-embedded weight contracts G (j-summed)
                    for h in range(2):
                        mm(bj(b_cur[:, ts(h, BC)]), s_wibr0[:, ts(h, S)],
                           gprev, start=False, stop=False)
                else:
                    red = wk.tile([1, BC], F32, tag="red")
                    src3 = gprev[0:1, :].rearrange("p (j b) -> p b j", j=3)
                    nc.vector.tensor_reduce(red[:], src3, mybir.AxisListType.X,
                                            ALU.add)
                    errt = wk.tile([1, BC], F16, tag="errt")
                    nc.vector.tensor_tensor(errt[:], red[:],
                                            s_negy[:, ts(t - 1, BC)], ALU.add)
                    for h in range(2):
                        mm(b_cur[:, ts(h, BC)], s_wib[:, ts(h, S)], errt[:],
                           start=False, stop=False)

            # ---- [ACT] A_enc tanh (spine) ----
            A_enc = wk.tile([S, BC], F16, tag="A_enc")
            nc.scalar.activation(A_enc[:], enc_cur, AF.Tanh)

            # ---- [PE] fillers while A_enc tanh runs ----
            if not first:
                if KRED:
                    for k in range(3):
                        mm(ns_cur[:, ts(k, BC)], s_whcat[:, ts(k, S)],
                           kprev[:], start=False, stop=False)
                elif FUSE_J:
                    for k in range(3):
                        mm(bj(ns_cur[:, ts(k, BC)]), s_whcat[:, ts(k, S)],
                           gprev, start=False, stop=False)
                else:
                    for k in range(3):
                        for j in range(3):
                            mm(ns_cur[:, ts(k, BC)], s_whcat[:, ts(k, S)],
                               gprev[:, ts(j, BC)], start=False, stop=False)
            if not last:
                xt1 = s_xw[:, ts(t + 1, BC)]
                eg_next = pEnc.tile([S, 512], F32, tag="a_enc")
                enc_next = eg_next[:, 0:BC]
                g_next = eg_next[:, BC:BC + GW]
                mm(enc_next, s_we[:], xt1, start=True, stop=False)
                ns_next = pNs.tile([S, 512], F32, name="a_ns", tag="a_ns")[:, 0:96]
                for k in range(3):
                    mm(ns_next[:, ts(k, BC)], s_wxcat[:, ts(k, S)], xt1,
                       start=(k == 0), stop=False)
            junk(JUNK1)

            # ---- [PE] wit (spine; waits A_enc) ----
            for h in range(2):
                mm(b_cur[:, ts(h, BC)], s_wit[h][:], A_enc[:],
                   start=(first and h == 0), stop=(h == 1))
            # zero-weight matmul: delays a_ns completion (hence A_ns tanh
            # readiness) until after A_enc/wit so the ACT scoreboard runs the
            # spine cst tanh before the off-spine A_ns tanh.
            mm(ns_cur[0:1, 0:BC], s_z1[:], A_enc[:], start=False, stop=True)
            # warm the PE pipe for the head folds during cst tanh
            junk(JUNK2, rhs1=A_enc[:, 0:1])

            # ---- [ACT] cst tanh (spine) ----
            s_cst = s_cst2[t % 2]
            nc.scalar.activation(s_cst[:, 0:2 * BC], b_cur[:], AF.Tanh)

            # ---- [PE] gate head (spine) ----
            mm(g_cur[:, 0:4], s_cst[:, 0:4 * BC], s_wfold[:, 0:4],
               start=False, stop=False)
            mm(g_cur[:, 0:4], s_cst[:, BC:5 * BC], s_wfold[:, 4:8],
               start=False, stop=True)

            # ---- [PE] fillers for t+1 during the gate phase ----
            if not last:
                b_full_next = pB.tile([S, 512], F32, name="b_", tag="b_")
                b_next = b_full_next[:, 0:2 * BC]
                if ERR_PE:
                    # -Wib*y_t rank-1 (err_t = pred_t - y_t for step t+1).
                    # negy is always-ready so it can pass the cst-blocked whc
                    # matmuls in the scoreboard: it must carry the bank's
                    # start=True, not whc00.
                    for h in range(2):
                        mm(b_next[:, ts(h, BC)], s_wib[:, ts(h, S)],
                           s_negy[:, ts(t, BC)], start=(h == 0), stop=False)
                for h in range(2):
                    mm(b_next[:, ts(h, BC)], s_whc[0][h][:], s_cst[:, 0:BC],
                       start=(h == 0 and not ERR_PE), stop=False)
                    mm(b_next[:, ts(h, BC)], s_whc[1][h][:],
                       s_cst[:, BC:2 * BC], start=False, stop=False)
                mm(g_next, s_ones132[:], s_bdext[:], start=False, stop=False)

            # ---- [ACT] exp (spine), A_ns tanh (off-spine) ----
            if GATE_B:
                # g region is 6 wide: [l0,l1,l2, out3, 0, -big] so one exp
                # gives e6 = [e0,e1,e2, q, 1, 0] and one grouped reduce gives
                # [z, w] = [e0+e1+e2, 1+q]; recip2 -> [rz, rw=1-theta].
                e = wk.tile([BC, 10], F32, tag="e")
                nc.scalar.activation(e[:, 0:6], g_cur[0:BC, 0:6], AF.Exp)
            else:
                e = wk.tile([BC, 3], F32, tag="e")
                if ACC_Z:
                    z = wk.tile([BC, 1], F32, tag="z")
                    nc.scalar.activation(e[:], g_cur[0:BC, 0:3], AF.Exp,
                                         accum_out=z[:])
                else:
                    nc.scalar.activation(e[:], g_cur[0:BC, 0:3], AF.Exp)
                th2 = wk.tile([BC, 1], F32, tag="th2")
                nc.scalar.activation(th2[:], g_cur[0:BC, 3:4], AF.Tanh)
            A_ns = wk.tile([S, 96], F16, tag="A_ns")
            nc.scalar.activation(A_ns[:], ns_cur[:], AF.Tanh)
            if DIAG_E:
                # PencT_j = A_ns_j^T @ Ue -> [32,128]x3 in b-bank cols 64:448
                # (bank freshness: negy's start=True last step reset it)
                for j in range(3):
                    mm(b_full[0:BC, 2 * BC + j * E:2 * BC + (j + 1) * E],
                       A_ns[:, ts(j, BC)], s_ue[:],
                       start=False, stop=(j == 2), skip_group_check=True)
                nc.scalar.copy(s_pencT[:], b_full[0:BC, 2 * BC:2 * BC + 3 * E])
            # gate-phase junk: ready once A_ns lands, fills the PE gap between
            # the whc block and gb so gb hits a warm pipe
            junk(JUNK3, rhs1=A_ns[:, 0:1])

            # ---- [DVE] gate algebra (spine) ----
            if GATE_B:
                nc.vector.tensor_reduce(
                    e[:, 6:8], e[:, 0:6].rearrange("p (a b) -> p a b", a=2),
                    mybir.AxisListType.X, ALU.add)
                nc.vector.reciprocal(e[:, 8:10], e[:, 6:8])
                if DIAG2:
                    dnew = dpar[(t + 1) % 2]
                    nc.vector._custom_dve(
                        gdiag_op,
                        out=dnew[:],
                        in0=e[:, 0:3].unsqueeze(2).broadcast_to([BC, 3, BC]),
                        in1=dpar[t % 2][:],
                        s0=e[:, 8:9], s1=e[:, 9:10])
                else:
                    nc.vector._custom_dve(
                        gate_op, out=gn[0:BC, 0:3], in0=e[:, 0:3],
                        in1=gf2[t % 2][0:BC, 0:3],
                        s0=e[:, 8:9], s1=e[:, 9:10])
            elif DS_PACK:
                if not ACC_Z:
                    z = wk.tile([BC, 1], F32, tag="z")
                    nc.vector.tensor_reduce(z[:], e[:], mybir.AxisListType.X,
                                            ALU.add)
                if not DIV_DS:
                    r0 = wk.tile([BC, 1], F32, tag="r0")
                    nc.vector.reciprocal(r0[:], z[:])
                # ds = [e/z - gate_prev | e/z + gate_prev]
                ds = wk.tile([BC, 6], F32, tag="ds")
                nc.vector.scalar_tensor_tensor(
                    ds[:].rearrange("p (a b) -> p a b", a=2),
                    e[:].unsqueeze(1).broadcast_to([BC, 2, 3]),
                    z[:] if DIV_DS else r0[:],
                    hgp2[t % 2][:].rearrange("p (a b) -> p a b", a=2),
                    ALU.divide if DIV_DS else ALU.mult, ALU.add)
                nc.vector.scalar_tensor_tensor(
                    gn[0:BC, 0:3], ds[:, 0:3], th2[:], ds[:, 3:6],
                    ALU.mult, ALU.add)
            else:
                dd = wk.tile([BC, 3], F32, tag="dd")
                nc.vector.scalar_tensor_tensor(
                    dd[:], e[:], r0[:], hgp2[t % 2][:, 0:3],
                    ALU.mult, ALU.add)
                ss = wk.tile([BC, 3], F32, tag="ss")
                nc.vector.scalar_tensor_tensor(
                    ss[:], e[:], r0[:], hgp2[t % 2][:, 3:6],
                    ALU.mult, ALU.add)
                nc.vector.scalar_tensor_tensor(
                    gn[0:BC, 0:3], dd[:], th2[:], ss[:], ALU.mult, ALU.add)

            # ---- [DVE] diag96 + [PE] enc diag-mms (spine when DIAG_E) ----
            if DIAG2:
                diag = dnew
            elif DIAG_E:
                diag = wk.tile([BC, 96], F16, tag="diag")
                nc.vector.tensor_tensor(
                    diag[:].rearrange("p (j b) -> p j b", j=3),
                    s_eyem[:].rearrange("p (j b) -> p j b", j=3),
                    gn[0:BC, 0:3].unsqueeze(2).broadcast_to([BC, 3, BC]),
                    ALU.mult)
            if DIAG_E and not last:
                for j in range(3):
                    mm(enc_next, s_pencT[:, ts(j, E)], diag[:, ts(j, BC)],
                       start=False, stop=False)

            # ---- [PE] gate transpose+broadcast (off-spine when DIAG_E) ----
            gb = pGB.tile([S, 512], F32, name="gb", tag="gb")[:, 0:96]
            if DIAG2:
                mm(gb[:], s_one32[:].broadcast_to([BC, S]), diag[:],
                   start=True, stop=True)
                junk(JUNK4, rhs1=diag[:, 0:1])
            else:
                for j in range(3):
                    mm(gb[:, ts(j, BC)],
                       gn[0:BC, j:j + 1].broadcast_to([BC, S]),
                       s_eye[0:BC, :], start=(j == 0), stop=(j == 2))
                junk(JUNK4, rhs1=gn[:, 0:1])

            # ---- [DVE] G = A_ns * gateB (spine) + hgp2 for t+1 ----
            nc.vector.tensor_mul(s_gall[:, ts(t, 96)], A_ns[:], gb[:])
            if KRED:
                kprev = wk.tile([S, BC], F16, tag="kst")
                with nc.allow_low_precision(reason="3-term f16 j-sum of f16 G"):
                    nc.vector.tensor_reduce(
                        kprev[:],
                        s_gall[:, ts(t, 96)].rearrange("p (j b) -> p b j", j=3),
                        mybir.AxisListType.X, ALU.add)
            if not last and not GATE_B:
                nc.vector.tensor_tensor(
                    hgp2[(t + 1) % 2][:].rearrange("p (a b) -> p a b", a=2),
                    gn[0:BC, 0:3].unsqueeze(1).broadcast_to([BC, 2, 3]),
                    c3, ALU.mult)

            enc_cur, ns_cur, b_cur, g_cur = (
                (None, None, None, None) if last
                else (enc_next, ns_next, b_next, g_next))
            if not last:
                b_full = b_full_next

        nc.sync.dma_start(out=g127o, in_=s_gall[0:1, :])
        stk.close()
    nc.finalize()
    return nc


# ---------------- host side ----------------

def _pack_inputs(x, y, Wx, Wh, We, Ue, Wi, Whc, Wo, Wd, bd, gate0, Tn=T):
    """Build the 8 per-core input dicts."""
    f16 = np.float16
    th_sc = 1.0 if GATE_B else 0.5
    F = np.concatenate(
        [Wo[:, :3] @ Wd, th_sc * Wo[:, 3:4]], axis=1
    ).astype(np.float32)  # [256, 4]
    # permute the S dim so the prediction feature (s=127) sits on partition 0
    # (matmul operands must have base partition 0/32/64)
    perm = np.arange(S)
    perm[[0, S - 1]] = [S - 1, 0]
    Whp = [Wh[j][perm][:, perm] for j in range(3)]
    Wxp = [Wx[j][:, perm] for j in range(3)]

    def padk(a):  # zero-pad contraction dim to 128 rows (FWL eligibility)
        out = np.zeros((S, a.shape[1]), np.float32)
        out[:a.shape[0]] = a
        return out

    eye = np.zeros((S, BC), np.float32)
    eye[0:BC, 0:BC] = (1.0 if GATE_B else 0.5) * np.eye(BC)
    wibr0 = np.zeros((S, H), np.float32)
    wibr0[0] = Wi[E]
    shared = {
        "whcat": np.concatenate(Whp, axis=1).astype(f16),
        "wxcat": padk(np.concatenate(Wxp, axis=1)).astype(f16),
        "ue": Ue[perm, :].astype(f16),
        "we": padk(We).astype(f16),
        "wit0": Wi[0:E, 0:S].astype(f16),
        "wit1": Wi[0:E, S:2 * S].astype(f16),
        "wib": Wi[E:E + 1].astype(f16),
        "wibr0": wibr0.astype(f16),
        "whc00": Whc[0:S, 0:S].astype(f16),
        "whc01": Whc[0:S, S:2 * S].astype(f16),
        "whc10": Whc[S:2 * S, 0:S].astype(f16),
        "whc11": Whc[S:2 * S, S:2 * S].astype(f16),
        "wfold": np.concatenate([F[0:S], F[S:2 * S]], axis=1).astype(f16),
        "bdext": (
            np.concatenate([bd, [0.0, 0.0, -30000.0]]).reshape(1, 6)
            if GATE_B else np.concatenate([bd, [0.0]]).reshape(1, 4)
        ).astype(f16),
        "eye32": eye.astype(f16),
        "ones132": np.concatenate(
            [np.ones((1, BC)), np.zeros((1, S - BC))], axis=1).astype(f16),
    }
    if DIAG_E:
        shared["eyem96"] = np.concatenate(
            [np.eye(BC)] * 3, axis=1).astype(f16)
    # per-core diag0 built in the core loop below
    in_maps = []
    for c in range(NCORES):
        bs = slice(c * BC, (c + 1) * BC)
        xs = x[bs, :Tn]                      # [32, T, 64]
        ys = y[bs, :Tn]                      # [32, T]
        g0 = gate0[bs]                       # [32, 3]
        gf20 = np.zeros((S, 32), np.float32)
        gf20[0:BC, 0:3] = (1.0 if GATE_B else 2.0) * g0
        xwp = np.zeros((S, Tn * BC), np.float32)
        xwp[0:D_IN] = xs.transpose(2, 1, 0).reshape(D_IN, Tn * BC)
        m = dict(shared)
        m["xw"] = xwp.astype(f16)
        m["negy"] = np.ascontiguousarray(
            (-ys.T).reshape(1, Tn * BC)
        ).astype(f16)
        m["gf20"] = gf20.astype(f16)
        if DIAG2:
            m["diag0"] = np.concatenate(
                [np.diag(g0[:, j]) for j in range(3)], axis=1).astype(f16)
        in_maps.append(m)
    return in_maps


_PROG_CACHE = {}
LAST_RESULT = {}


def kernel(x, y, Wx, Wh, We, Ue, Wi, Whc, Wo, Wd, bd, gate0):
    from concourse.bass_utils import run_bass_kernel_spmd

    args = [np.asarray(a, dtype=np.float32) for a in
            (x, y, Wx, Wh, We, Ue, Wi, Whc, Wo, Wd, bd, gate0)]
    in_maps = _pack_inputs(*args)
    if "prog" not in _PROG_CACHE:
        _PROG_CACHE["prog"] = build_program(T)
    nc = _PROG_CACHE["prog"]
    trace = bool(int(os.environ.get("TRN_KERNEL_TRACE", "0")))
    res = run_bass_kernel_spmd(
        nc, in_maps, core_ids=list(range(NCORES)), trace=trace
    )
    LAST_RESULT["exec_time_ns"] = res.exec_time_ns
    LAST_RESULT["res"] = res
    preds = np.zeros((B, T), np.float32)
    for c in range(NCORES):
        g127 = res.results[c]["g127"].reshape(T, 3, BC).astype(np.float32)
        preds[c * BC:(c + 1) * BC] = g127.sum(axis=1).T
    return preds

